# revision 1
# baseline (speedup 1.0000x reference)
"""Trainium2 Bass kernel for nn_CHAN_without_SA (conv/attention/deconv scorer).

Full-input contract: kernel(**inputs) takes the complete unsharded inputs,
shards data-parallel over batch*max_seg_num across 8 NeuronCores (10 sequences
per core; each core's sequences all belong to one batch element), runs one SPMD
Bass/Tile program, and reassembles the full output.

Device program per core (all matmuls bf16 with fp32 PSUM accumulation):
  conv1 (k=5, 2048->512) + maxpool2 : tap-accumulated shifted-window matmuls
  conv2 (k=5,  512->256) + maxpool2 : same
  additive attention x2 concepts    : kproj matmul, tanh(+q), score matmul,
                                      masked softmax, weighted sum via DVE
  deconvs (k=4,s=2,p=1) x2          : even/odd output-phase matmuls
  final score                       : folded projection  sigmoid(v . d2 + b)
where v = w_sim1^T ((w_sim2 @ concept) * w_mlp[0]) collapses the SDIM=1024
projection exactly (algebraic identity, per batch element).
"""
import numpy as np
import ml_dtypes

BF16 = ml_dtypes.bfloat16

B, M, L = 4, 20, 200
IN_C, C1, C2 = 2048, 512, 256
CDIM, DC1, DC2, SDIM = 300, 512, 256, 1024
NEG = -1e15
Lq = L // 4           # 50
NCORES = 8
SEQ = 10              # sequences per core
PAIRS = 5
K1, K2 = 16, 4        # contraction tiles for conv1 (2048/128) and conv2 (512/128)
M1, M2 = 4, 2         # output tiles for conv1 (512/128) and conv2 (256/128)
KD1, MD1 = 6, 4       # deconv1: 768/128 in, 512/128 out
KD2, MD2 = 4, 2       # deconv2: 512/128 in, 256/128 out


def _build_program():
    import concourse.bass as bass
    import concourse.mybir as mybir
    import concourse.tile as tile
    from concourse import bacc
    from contextlib import ExitStack

    dt = mybir.dt
    f32, bf16 = dt.float32, dt.bfloat16
    AF = mybir.ActivationFunctionType
    ALU = mybir.AluOpType

    nc = bacc.Bacc()
    P = nc.declare_dram_parameter
    d_xw = P("xw", [PAIRS * 8, 128, 816], bf16, isOutput=False)
    d_w1 = P("w1t", [K1, 128, 5 * 512], bf16, isOutput=False)
    d_b1 = P("b1", [M1, 128, 1], f32, isOutput=False)
    d_w2 = P("w2t", [K2, 128, 5 * 256], bf16, isOutput=False)
    d_b2 = P("b2", [M2, 128, 1], f32, isOutput=False)
    d_wca2 = P("wca2t", [2, 128, 256], bf16, isOutput=False)
    d_wca3 = P("wca3t", [2, 128, 1], bf16, isOutput=False)
    d_qv = P("qv", [4, 128, 1], f32, isOutput=False)
    d_mask = P("amask", [PAIRS, 1, 2 * Lq], f32, isOutput=False)
    d_wd1 = P("wd1t", [KD1, 128, 4 * 512], bf16, isOutput=False)
    d_bd1 = P("bd1", [MD1, 128, 1], f32, isOutput=False)
    d_wd2 = P("wd2t", [KD2, 128, 4 * 256], bf16, isOutput=False)
    d_bd2 = P("bd2", [MD2, 128, 1], f32, isOutput=False)
    d_wsum = P("wsum", [8, 128, 512], bf16, isOutput=False)
    d_v = P("vmat", [2, 128, 2], bf16, isOutput=False)
    d_bmlp = P("bmlp", [2, 1], f32, isOutput=False)
    d_out = P("out", [2, SEQ * L], f32, isOutput=True)

    with ExitStack() as ctx:
        tc = ctx.enter_context(tile.TileContext(nc))
        wp = ctx.enter_context(tc.tile_pool(name="weights", bufs=1))
        ap_ = ctx.enter_context(tc.tile_pool(name="acts", bufs=1))
        tp = ctx.enter_context(tc.tile_pool(name="trans", bufs=2))
        pp = ctx.enter_context(tc.tile_pool(name="psum", bufs=1, space="PSUM"))
        dp = ctx.enter_context(tc.tile_pool(name="drambounce", bufs=1, space="DRAM"))

        _eng_ctr = [0]

        def bulk_eng():
            # alternate the two HWDGE issue engines (SP / ACT) so bulk loads
            # use both hardware queue sets in parallel
            _eng_ctr[0] += 1
            return nc.sync if _eng_ctr[0] % 2 == 0 else nc.scalar

        def wtile(src, i, shape, dtyp, tag, small=False):
            t = wp.tile(shape, dtyp, tag=tag, name=tag)
            # small constants go via SWDGE: one queue sem per DMA, so their
            # consumers (ACT/DVE) don't blow the per-inst sync-wait budget
            eng = nc.gpsimd if small else bulk_eng()
            eng.dma_start(out=t, in_=src[i] if i is not None else src[:])
            return t

        def xtile(p, k2):
            # two conv1 k-tiles per DMA: 1632B per-partition chunks
            xk = tp.tile([128, 816], bf16, tag=f"x{k2}", name=f"x{k2}", bufs=2)
            bulk_eng().dma_start(out=xk, in_=d_xw[p * 8 + k2])
            return xk

        # ---- DMA issue order is the kernel head's critical path: per k-step
        # emit the pair-0/pair-1 x tiles around their 5-tap conv1 weight tile
        # (exact consumption order) so the conv1 passes stream behind the DMA.
        xs_pre = {0: [None] * 8, 1: [None] * 8}
        w1k = [None] * K1
        w1s = {}
        for k2 in range(8):
            xs_pre[0][k2] = xtile(0, k2)
            for k in (2 * k2, 2 * k2 + 1):
                if k < 2:
                    # per-tap loads for the first k-tiles: the very first
                    # matmul only has to wait for a 128KB transfer
                    w1s[k] = []
                    for t in range(5):
                        wt = wp.tile([128, 512], bf16, tag=f"w1s_{k}_{t}",
                                     name=f"w1s_{k}_{t}")
                        bulk_eng().dma_start(
                            out=wt, in_=d_w1[k, :, t * 512:(t + 1) * 512])
                        w1s[k].append(wt)
                else:
                    w1k[k] = wtile(d_w1, k, [128, 5 * 512], bf16, f"w1k_{k}")
            xs_pre[1][k2] = xtile(1, k2)
        w2k = [wtile(d_w2, k, [128, 5 * 256], bf16, f"w2k_{k}") for k in range(K2)]
        wca2 = [wtile(d_wca2, k, [128, 256], bf16, f"wca2_{k}") for k in range(2)]
        wca3 = [wtile(d_wca3, k, [128, 1], bf16, f"wca3_{k}", small=True) for k in range(2)]
        qv = [[wtile(d_qv, c * 2 + k, [128, 1], f32, f"qv_{c}_{k}", small=True)
               for k in range(2)] for c in range(2)]
        b1 = [wtile(d_b1, m, [128, 1], f32, f"b1_{m}", small=True) for m in range(M1)]
        b2 = [wtile(d_b2, m, [128, 1], f32, f"b2_{m}", small=True) for m in range(M2)]
        bd1 = [wtile(d_bd1, m, [128, 1], f32, f"bd1_{m}", small=True) for m in range(MD1)]
        bd2 = [wtile(d_bd2, m, [128, 1], f32, f"bd2_{m}", small=True) for m in range(MD2)]
        vm = [wtile(d_v, k, [128, 2], bf16, f"v_{k}", small=True) for k in range(2)]
        bmlp = wtile(d_bmlp, None, [2, 1], f32, "bmlp", small=True)
        mkp = [wtile(d_mask, p, [1, 2 * Lq], f32, f"mask{p}", small=True)
               for p in range(PAIRS)]

        # ---- persistent activation tiles ----
        # cat: [t2(2) | r1(2) | r2(2)] k-tiles, 10 seqs x 52 cols (1 zero pad each side)
        cat = [ap_.tile([128, SEQ * 52], bf16, tag=f"cat{j}", name=f"cat{j}") for j in range(2)]
        for t in cat:
            nc.gpsimd.memset(t, 0.0)
        d1p = [ap_.tile([128, SEQ * 102], bf16, tag=f"d1p{m}", name=f"d1p{m}") for m in range(MD1)]
        for t in d1p:
            nc.gpsimd.memset(t, 0.0)
        d2sb = [ap_.tile([128, SEQ * 200], bf16, tag=f"d2_{m}", name=f"d2_{m}") for m in range(MD2)]
        rcol = [[ap_.tile([128, SEQ], f32, tag=f"rcol{c}{k}", name=f"rcol{c}{k}") for k in range(2)]
                for c in range(2)]

        wd1k = [None] * KD1
        wd2k = [None] * KD2
        wsum = [None] * 8

        # ============ conv + attention (incl. softmax + r), per pair ========
        for p in range(PAIRS):
            if p >= 1 and p + 1 < PAIRS:
                xs_pre[p + 1] = [xtile(p + 1, k2) for k2 in range(8)]
            if p == 2:
                # deconv weights are needed only after the conv phase; issue
                # them here so they queue behind the x prefetches they'd
                # otherwise starve
                for k in range(KD1):
                    wd1k[k] = wtile(d_wd1, k, [128, 4 * 512], bf16, f"wd1k_{k}")
                for k in range(KD2):
                    wd2k[k] = wtile(d_wd2, k, [128, 4 * 256], bf16, f"wd2k_{k}")
                for j in range(8):
                    wsum[j] = wtile(d_wsum, j, [128, 512], bf16, f"wsum_{j}")
            xs = xs_pre[p]

            # conv1: k,t outer / m inner -> each weight tile is consumed for
            # all 4 output tiles as soon as it lands (head-of-kernel overlap)
            psg = [pp.tile([128, 400], f32, tag="mm400", name="mm400", bufs=4)
                   for _ in range(M1)]
            n = 0
            for k in range(K1):
                k2, k01 = divmod(k, 2)
                rv = xs[k2].rearrange("q (g s c) -> q g s c", g=2, s=2)
                for t in range(5):
                    lh = (w1s[k][t][:, :] if k < 2 else
                          w1k[k][:, t * 512:(t + 1) * 512])
                    for m in range(M1):
                        nc.tensor.matmul(
                            psg[m], lhsT=lh[:, m * 128:(m + 1) * 128],
                            rhs=rv[:, k01, :, t:t + 200],
                            start=(n == 0), stop=(n == 5 * K1 - 1))
                    n += 1
            t1 = []
            for m in range(M1):
                ps = psg[m]
                t1m = tp.tile([128, 2 * 104], bf16, tag=f"t1_{m}", name=f"t1_{m}", bufs=2)
                tmp = tp.tile([128, 200], f32, tag="ptmp1", name="ptmp1", bufs=3)
                pr = ps.rearrange("q (s l two) -> q s l two", s=2, two=2)
                tv = tmp.rearrange("q (s l) -> q s l", s=2)
                # pool+bias: max(even+b, odd+b); only one PSUM input per inst
                nc.scalar.activation(out=tv, in_=pr[:, :, :, 0],
                                     func=AF.Identity, bias=b1[m], scale=1.0)
                nc.gpsimd.memset(t1m, 0.0)
                nc.vector.scalar_tensor_tensor(
                    out=t1m.rearrange("q (s c) -> q s c", s=2)[:, :, 2:102],
                    in0=pr[:, :, :, 1], scalar=b1[m], in1=tv,
                    op0=ALU.add, op1=ALU.max)
                t1.append(t1m)

            # conv2 + pool -> t2 part of cat
            for m in range(M2):
                ps = pp.tile([128, 200], f32, tag="mm200", name="mm200", bufs=2)
                n = 0
                for k in range(K2):
                    rv = t1[k].rearrange("q (s c) -> q s c", s=2)
                    for t in range(5):
                        nc.tensor.matmul(
                            ps,
                            lhsT=w2k[k][:, t * 256 + m * 128:t * 256 + (m + 1) * 128],
                            rhs=rv[:, :, t:t + 100],
                            start=(n == 0), stop=(n == 5 * K2 - 1))
                        n += 1
                tmp = tp.tile([128, 100], f32, tag="ptmp2", name="ptmp2", bufs=3)
                pr = ps.rearrange("q (s l two) -> q s l two", s=2, two=2)
                tv = tmp.rearrange("q (s l) -> q s l", s=2)
                nc.scalar.activation(out=tv, in_=pr[:, :, :, 0],
                                     func=AF.Identity, bias=b2[m], scale=1.0)
                nc.vector.scalar_tensor_tensor(
                    out=cat[m].rearrange("q (s c) -> q s c", s=SEQ)[
                        :, 2 * p:2 * p + 2, 1:1 + Lq],
                    in0=pr[:, :, :, 1], scalar=b2[m], in1=tv,
                    op0=ALU.add, op1=ALU.max)

            # attention for this pair (runs on ACT/DVE/DMA under the next
            # pair's conv1 on PE)
            kp = []
            for m in range(M2):
                kpm = pp.tile([128, 100], f32, tag="mm200", name="mm200", bufs=2)
                for k in range(2):
                    nc.tensor.matmul(
                        kpm, lhsT=wca2[k][:, m * 128:(m + 1) * 128],
                        rhs=cat[k].rearrange("q (s c) -> q s c", s=SEQ)[
                            :, 2 * p:2 * p + 2, 1:1 + Lq],
                        start=(k == 0), stop=(k == 1))
                kp.append(kpm)
            for c in range(2):
                th = []
                for m in range(M2):
                    thm = tp.tile([128, 100], bf16, tag=f"th{c}{m}", name=f"th{c}{m}", bufs=2)
                    nc.scalar.activation(out=thm, in_=kp[m], func=AF.Tanh,
                                         bias=qv[c][m], scale=1.0)
                    th.append(thm)
                sp = pp.tile([1, 100], f32, tag="tiny", name="tiny", bufs=1)
                for m in range(M2):
                    nc.tensor.matmul(sp, lhsT=wca3[m], rhs=th[m],
                                     start=(m == 0), stop=(m == 1))
                # masked softmax in flat [1, 100] layout (2 blocks of 50);
                # per-block broadcasts use 0-stride AP reads on DVE
                def bc2(t):
                    return bass.AP(tensor=t.tensor, offset=t.offset,
                                   ap=[t.ap[0], [1, 2], [0, Lq]])
                sfl = tp.tile([1, 100], f32, tag="sfl", name="sfl", bufs=4)
                nc.vector.tensor_copy(out=sfl, in_=sp[0:1, 0:100])
                sm = tp.tile([1, 100], f32, tag="sm", name="sm", bufs=4)
                nc.vector.tensor_add(sm, sfl, mkp[p])
                smv = sm.rearrange("q (s l) -> q s l", s=2)
                mx = tp.tile([1, 2], f32, tag="mx", name="mx", bufs=4)
                nc.vector.tensor_reduce(out=mx, in_=smv,
                                        axis=mybir.AxisListType.X, op=ALU.max)
                sub = tp.tile([1, 100], f32, tag="sub", name="sub", bufs=4)
                nc.vector.tensor_sub(sub.rearrange("q (s l) -> q s l", s=2),
                                     smv, bc2(mx))
                ex = tp.tile([1, 100], f32, tag="ex", name="ex", bufs=4)
                nc.scalar.activation(out=ex, in_=sub, func=AF.Exp,
                                     bias=0.0, scale=1.0)
                exv = ex.rearrange("q (s l) -> q s l", s=2)
                se = tp.tile([1, 2], f32, tag="se", name="se", bufs=4)
                nc.vector.tensor_reduce(out=se, in_=exv,
                                        axis=mybir.AxisListType.X, op=ALU.add)
                rc = tp.tile([1, 2], f32, tag="rc", name="rc", bufs=4)
                nc.vector.reciprocal(rc, se)
                av = tp.tile([1, 100], f32, tag="av", name="av", bufs=4)
                nc.vector.tensor_mul(av.rearrange("q (s l) -> q s l", s=2),
                                     exv, bc2(rc))
                # broadcast attention weights to all partitions (no DRAM hop)
                abc2 = tp.tile([128, 100], f32, tag="abc", name="abc", bufs=4)
                nc.gpsimd.partition_broadcast(abc2, av[0:1, :], channels=128)
                for s01 in range(2):
                    s = 2 * p + s01
                    abc = abc2[:, Lq * s01:Lq * s01 + Lq]
                    for k in range(2):
                        scr = tp.tile([128, Lq], f32, tag="rscr", name="rscr", bufs=2)
                        nc.vector.tensor_mul(
                            scr, cat[k][:, 52 * s + 1:52 * s + 1 + Lq], abc)
                        nc.vector.tensor_reduce(
                            out=rcol[c][k][:, s:s + 1], in_=scr,
                            axis=mybir.AxisListType.X, op=ALU.add)

        # ================= deconv1: cat[768,50] -> d1[512,100] ==============
        # even out o=2j: tap1@U[j] + tap3@U[j-1]; odd o=2j+1: tap2@U[j] + tap0@U[j+1]
        # r-channels (256..767) are constant along j, so their contribution is
        # rank-1 per sequence: broadcast((Wt_a+Wt_b)^T r) with a single-column
        # boundary correction (j=0 even: -W3^T r; j=49 odd: -W0^T r).
        rcolb = []
        for c in range(2):
            for k in range(2):
                rb = ap_.tile([128, SEQ], bf16, tag=f"rcolb{c}{k}",
                              name=f"rcolb{c}{k}")
                nc.vector.tensor_copy(out=rb, in_=rcol[c][k])
                rcolb.append(rb)          # rk = 2*c + k matches cat channel order
        rt = pp.tile([128, 320], f32, tag="rt", name="rt", bufs=1)
        for phase in range(2):
            corr_t = 3 if phase == 0 else 0
            for m in range(MD1):
                g = phase * 4 + m
                for rk in range(4):
                    nc.tensor.matmul(
                        rt[:, g * 20:g * 20 + 10],
                        lhsT=wsum[phase * 4 + rk][:, m * 128:(m + 1) * 128],
                        rhs=rcolb[rk], start=(rk == 0), stop=(rk == 3))
                for rk in range(4):
                    nc.tensor.matmul(
                        rt[:, g * 20 + 10:g * 20 + 20],
                        lhsT=wd1k[2 + rk][:, corr_t * 512 + m * 128:
                                          corr_t * 512 + (m + 1) * 128],
                        rhs=rcolb[rk], start=(rk == 0), stop=(rk == 3))
        for m in range(MD1):
            for phase, taps in enumerate([[(1, 1), (3, 0)], [(2, 1), (0, 2)]]):
                ps = pp.tile([128, SEQ * Lq], f32, tag="mm400", name="mm400", bufs=4)
                n = 0
                for k in range(2):
                    for t, off in taps:
                        nc.tensor.matmul(
                            ps,
                            lhsT=wd1k[k][:, t * 512 + m * 128:t * 512 + (m + 1) * 128],
                            rhs=cat[k].rearrange("q (s c) -> q s c", s=SEQ)[
                                :, :, off:off + Lq],
                            start=(n == 0), stop=(n == 3))
                        n += 1
                g = phase * 4 + m
                ert = tp.tile([128, 20], f32, tag="ert", name="ert", bufs=2)
                nc.vector.tensor_copy(out=ert, in_=rt[:, g * 20:g * 20 + 20])
                er2 = tp.tile([128, SEQ], f32, tag="er2", name="er2", bufs=2)
                nc.vector.tensor_sub(er2, ert[:, 0:10], ert[:, 10:20])
                dst = d1p[m].rearrange("q (s c) -> q s c", s=SEQ)[:, :, 1:101] \
                    .rearrange("q s (l two) -> q s l two", two=2)[:, :, :, phase]
                erb = bass.AP(tensor=ert.tensor, offset=ert.offset,
                              ap=[ert.ap[0], [1, SEQ], [0, Lq]])
                nc.vector.scalar_tensor_tensor(
                    out=dst, in0=ps.rearrange("q (s l) -> q s l", s=SEQ),
                    scalar=bd1[m], in1=erb, op0=ALU.add, op1=ALU.add)
                bcol = 0 if phase == 0 else Lq - 1
                nc.vector.scalar_tensor_tensor(
                    out=dst[:, :, bcol:bcol + 1],
                    in0=ps.rearrange("q (s l) -> q s l", s=SEQ)[:, :, bcol:bcol + 1],
                    scalar=bd1[m],
                    in1=er2.rearrange("q (s o) -> q s o", o=1),
                    op0=ALU.add, op1=ALU.add)

        # ================= deconv2: d1[512,100] -> d2[256,200] ==============
        for m in range(MD2):
            for g in range(2):          # groups of 5 seqs (PSUM bank limit)
                for phase, taps in enumerate([[(1, 1), (3, 0)], [(2, 1), (0, 2)]]):
                    ps = pp.tile([128, 500], f32, tag="mm400", name="mm400", bufs=4)
                    n = 0
                    for k in range(KD2):
                        for t, off in taps:
                            nc.tensor.matmul(
                                ps,
                                lhsT=wd2k[k][:, t * 256 + m * 128:t * 256 + (m + 1) * 128],
                                rhs=d1p[k].rearrange("q (s c) -> q s c", s=SEQ)[
                                    :, 5 * g:5 * g + 5, off:off + 100],
                                start=(n == 0), stop=(n == 2 * KD2 - 1))
                            n += 1
                    dst = d2sb[m].rearrange("q (s c) -> q s c", s=SEQ)[
                        :, 5 * g:5 * g + 5, :] \
                        .rearrange("q s (l two) -> q s l two", two=2)[:, :, :, phase]
                    nc.scalar.activation(
                        out=dst, in_=ps.rearrange("q (s l) -> q s l", s=5),
                        func=AF.Identity, bias=bd2[m], scale=1.0)

        # ================= folded final projection + sigmoid ================
        for p in range(PAIRS):
            fp = pp.tile([2, 400], f32, tag="tiny", name="tiny", bufs=1)
            for k in range(2):
                nc.tensor.matmul(fp, lhsT=vm[k],
                                 rhs=d2sb[k][:, 400 * p:400 * (p + 1)],
                                 start=(k == 0), stop=(k == 1))
            fo = tp.tile([2, 400], f32, tag="fout", name="fout", bufs=5)
            nc.scalar.activation(out=fo, in_=fp, func=AF.Sigmoid,
                                 bias=bmlp, scale=1.0)
            nc.sync.dma_start(out=d_out[:, 400 * p:400 * (p + 1)], in_=fo)

    nc.compile()   # bacc legalization: splits sync waits to <=1 per inst
    return nc


def _prep_inputs(batch, seg_len, concept1, concept2,
                 w_conv1, b_conv1, w_conv2, b_conv2,
                 w_ca1, w_ca2, w_ca3,
                 w_dc1, b_dc1, w_dc2, b_dc2,
                 w_sim1, w_sim2, w_mlp, b_mlp):
    f32 = np.float32

    # x: [B,M,L,IN_C] -> per core [PAIRS*K1, 128, 408] padded pairs (bf16)
    bm = np.ascontiguousarray(batch, f32).reshape(B * M, L, IN_C)
    bt = bm.transpose(0, 2, 1).astype(BF16)            # [80, 2048, 200]
    X = np.zeros((B * M, K1, 128, 204), BF16)
    X[:, :, :, 2:202] = bt.reshape(B * M, K1, 128, L)
    # pack [pair, k2(8), 128, (k01, seq01, 204)] = [.., 128, 816]
    xw = X.reshape(NCORES, PAIRS, 2, 8, 2, 128, 204) \
          .transpose(0, 1, 3, 5, 4, 2, 6) \
          .reshape(NCORES, PAIRS * 8, 128, 816)
    xw = np.ascontiguousarray(xw)

    # weight layouts: one DMA per contraction k-tile holding all taps
    # [k, ci, (t, m, co)] -> big per-partition contiguous chunks
    w1t = np.ascontiguousarray(
        np.asarray(w_conv1, f32).reshape(M1, 128, K1, 128, 5)
        .transpose(2, 3, 4, 0, 1).reshape(K1, 128, 5 * 512)).astype(BF16)
    w2t = np.ascontiguousarray(
        np.asarray(w_conv2, f32).reshape(M2, 128, K2, 128, 5)
        .transpose(2, 3, 4, 0, 1).reshape(K2, 128, 5 * 256)).astype(BF16)
    wd1t = np.ascontiguousarray(
        np.asarray(w_dc1, f32).reshape(KD1, 128, MD1, 128, 4)
        .transpose(0, 1, 4, 2, 3).reshape(KD1, 128, 4 * 512)).astype(BF16)
    wd2t = np.ascontiguousarray(
        np.asarray(w_dc2, f32).reshape(KD2, 128, MD2, 128, 4)
        .transpose(0, 1, 4, 2, 3).reshape(KD2, 128, 4 * 256)).astype(BF16)
    wca2t = np.ascontiguousarray(np.asarray(w_ca2, f32).T.reshape(2, 128, 256)) \
        .astype(BF16)
    # summed-tap deconv1 weights for the broadcast r-channels: [ph*4+rk, ci, (m co)]
    wr = np.asarray(w_dc1, f32)[256:768].reshape(4, 128, MD1, 128, 4)
    wsum = np.ascontiguousarray(
        np.stack([wr[..., 1] + wr[..., 3], wr[..., 2] + wr[..., 0]], 0)
        .reshape(8, 128, 512)).astype(BF16)
    wca3t = np.asarray(w_ca3, f32)[0].reshape(2, 128, 1).astype(BF16)
    b1 = np.asarray(b_conv1, f32).reshape(M1, 128, 1)
    b2 = np.asarray(b_conv2, f32).reshape(M2, 128, 1)
    bd1v = np.asarray(b_dc1, f32).reshape(MD1, 128, 1)
    bd2v = np.asarray(b_dc2, f32).reshape(MD2, 128, 1)
    bmlp = np.full((2, 1), np.asarray(b_mlp, f32).reshape(-1)[0], f32)

    # per-core mask / q / v
    nvalid = ((np.asarray(seg_len) + 3) // 4).reshape(B * M)
    amask = np.where(np.arange(Lq)[None, :] < nvalid[:, None], 0.0, NEG) \
        .astype(f32).reshape(NCORES, PAIRS, 1, 2 * Lq)
    concepts = [np.asarray(concept1, f32), np.asarray(concept2, f32)]
    w_ca1 = np.asarray(w_ca1, f32)
    w_sim1 = np.asarray(w_sim1, f32)
    w_sim2 = np.asarray(w_sim2, f32)
    wm = np.asarray(w_mlp, f32)[0]
    qv_all = np.zeros((NCORES, 4, 128, 1), f32)
    v_all = np.zeros((NCORES, 2, 128, 2), f32)
    for core in range(NCORES):
        bidx = (core * SEQ) // M
        for c in range(2):
            q = w_ca1 @ concepts[c][bidx]                       # [256]
            qv_all[core, 2 * c:2 * c + 2] = q.reshape(2, 128, 1)
            v = w_sim1.T @ ((w_sim2 @ concepts[c][bidx]) * wm)  # [256]
            v_all[core, :, :, c] = v.reshape(2, 128)
    vmat = v_all.astype(BF16)

    shared = dict(w1t=w1t, b1=b1, wsum=wsum, w2t=w2t, b2=b2, wca2t=wca2t, wca3t=wca3t,
                  wd1t=wd1t, bd1=bd1v, wd2t=wd2t, bd2=bd2v, bmlp=bmlp)
    return [dict(shared, xw=xw[c], amask=amask[c], qv=qv_all[c], vmat=vmat[c])
            for c in range(NCORES)]


_CACHE = {}


def kernel(**inputs):
    from concourse.bass_utils import run_bass_kernel_spmd

    in_maps = _prep_inputs(**inputs)
    if "nc" not in _CACHE:
        _CACHE["nc"] = _build_program()
    res = run_bass_kernel_spmd(_CACHE["nc"], in_maps, list(range(NCORES)))
    out = np.stack([np.asarray(r["out"], np.float32) for r in res.results])
    sc = out.transpose(1, 0, 2).reshape(2, B, M, L)
    return sc[0], sc[1]



# revision 5
# speedup vs baseline: 1.7805x; 1.7805x over previous
"""Trainium2 Bass kernel for nn_CHAN_without_SA (conv/attention/deconv scorer).

Full-input contract: kernel(**inputs) takes the complete unsharded inputs,
shards data-parallel over batch*max_seg_num across 8 NeuronCores (10 sequences
per core; each core's sequences all belong to one batch element), runs one SPMD
Bass/Tile program, and reassembles the full output.

Device program per core. The heavy matmuls (conv1/conv2/deconv1/deconv2/kproj)
run in fp8-e4m3 with DoubleRow perf mode: two 128-deep contraction tiles per
instruction, 2x PE throughput vs bf16. Weights are pre-scaled by 16 on the host
so their values sit in e4m3's normal range (min normal 2^-6); the 16x (or
16*16=256x for conv2, whose input t1 is itself stored at 16x) product scale is
divided back out in the fp32 activation stage that reads PSUM.
  conv1 (k=5, 2048->512) + maxpool2 : tap-accumulated shifted-window matmuls
  conv2 (k=5,  512->256) + maxpool2 : same
  additive attention x2 concepts    : kproj matmul, tanh(+q), score matmul,
                                      masked softmax, weighted sum via DVE
  deconvs (k=4,s=2,p=1) x2          : even/odd output-phase matmuls
  final score                       : folded projection  sigmoid(v . d2 + b)
where v = w_sim1^T ((w_sim2 @ concept) * w_mlp[0]) collapses the SDIM=1024
projection exactly (algebraic identity, per batch element).
"""
import numpy as np
import ml_dtypes

BF16 = ml_dtypes.bfloat16
E4M3 = ml_dtypes.float8_e4m3   # TRN fp8_e4m3: max normal 240

B, M, L = 4, 20, 200
IN_C, C1, C2 = 2048, 512, 256
CDIM, DC1, DC2, SDIM = 300, 512, 256, 1024
NEG = -1e15
Lq = L // 4           # 50
NCORES = 8
SEQ = 10              # sequences per core
PAIRS = 5
M1, M2 = 4, 2         # output tiles for conv1 (512/128) and conv2 (256/128)
MD1, MD2 = 4, 2       # deconv out tiles: 512/128 and 256/128
SW = 16.0             # fp8 weight pre-scale (power of 2)


def _q8(v):
    return np.clip(np.asarray(v, np.float32), -240, 240).astype(E4M3)


def _build_program():
    import concourse.bass as bass
    import concourse.mybir as mybir
    import concourse.tile as tile
    from concourse import bacc
    from contextlib import ExitStack

    dt = mybir.dt
    f32, bf16, fp8 = dt.float32, dt.bfloat16, dt.float8e4
    AF = mybir.ActivationFunctionType
    ALU = mybir.AluOpType
    DR = mybir.MatmulPerfMode.DoubleRow

    nc = bacc.Bacc()
    P = nc.declare_dram_parameter
    # x: per (pair, k4): [128, (k2b, g, s, 204)] fp8
    d_xw = P("xw", [PAIRS * 4, 128, 1632], fp8, isOutput=False)
    # conv1 weights (16x): per k2 group: [128, (g, t, m, co)] fp8
    d_w1 = P("w1t", [8, 128, 5120], fp8, isOutput=False)
    d_b1 = P("b1", [M1, 128, 1], f32, isOutput=False)        # 16x b_conv1
    # conv2 weights (16x): per group: [128, (g, t, m, co)] fp8
    d_w2 = P("w2t", [2, 128, 2560], fp8, isOutput=False)
    d_b2 = P("b2", [M2, 128, 1], f32, isOutput=False)
    d_wca2 = P("wca2t", [128, 512], fp8, isOutput=False)     # 16x, (g, co)
    d_wca3 = P("wca3t", [2, 128, 1], bf16, isOutput=False)
    d_qv = P("qv", [4, 128, 1], f32, isOutput=False)
    d_mask = P("amask", [PAIRS, 1, 2 * Lq], f32, isOutput=False)
    # deconv1 t2-part weights (16x): [128, (g, t, m, co)] fp8
    d_wd1 = P("wd1t", [128, 4096], fp8, isOutput=False)
    d_bd1 = P("bd1", [MD1, 128, 1], f32, isOutput=False)
    # deconv1 r-part boundary-correction taps (bf16, true scale)
    d_wcorr = P("wcorr", [8, 128, 512], bf16, isOutput=False)
    # deconv2 weights (16x): per kg: [128, (g, t, m, co)] fp8
    d_wd2 = P("wd2t", [2, 128, 2048], fp8, isOutput=False)
    d_bd2 = P("bd2", [MD2, 128, 1], f32, isOutput=False)
    d_wsum = P("wsum", [8, 128, 512], bf16, isOutput=False)
    d_v = P("vmat", [2, 128, 2], bf16, isOutput=False)
    d_bmlp = P("bmlp", [2, 1], f32, isOutput=False)
    d_out = P("out", [2, SEQ * L], f32, isOutput=True)

    with ExitStack() as ctx:
        tc = ctx.enter_context(tile.TileContext(nc))
        wp = ctx.enter_context(tc.tile_pool(name="weights", bufs=1))
        ap_ = ctx.enter_context(tc.tile_pool(name="acts", bufs=1))
        tp = ctx.enter_context(tc.tile_pool(name="trans", bufs=2))
        pp = ctx.enter_context(tc.tile_pool(name="psum", bufs=1, space="PSUM"))

        _eng_ctr = [0]

        def bulk_eng():
            # alternate the two HWDGE issue engines (SP / ACT) so bulk loads
            # use both hardware queue sets in parallel
            _eng_ctr[0] += 1
            return nc.sync if _eng_ctr[0] % 2 == 0 else nc.scalar

        def wtile(src, i, shape, dtyp, tag, small=False):
            t = wp.tile(shape, dtyp, tag=tag, name=tag)
            # small constants go via SWDGE: one queue sem per DMA, so their
            # consumers (ACT/DVE) don't blow the per-inst sync-wait budget
            eng = nc.gpsimd if small else bulk_eng()
            eng.dma_start(out=t, in_=src[i] if i is not None else src[:])
            return t

        def xtile(p, k4):
            xk = tp.tile([128, 1632], fp8, tag=f"x{k4}", name=f"x{k4}", bufs=2)
            bulk_eng().dma_start(out=xk, in_=d_xw[p * 4 + k4])
            return xk

        # ---- DMA issue order is the kernel head's critical path: interleave
        # the pair-0/pair-1 x tiles with the conv1 weight groups in exact
        # consumption order so the conv1 passes stream behind the DMA.
        xs_pre = {0: [None] * 4, 1: [None] * 4}
        w1g = [None] * 8
        w1s = []
        for k2 in range(8):
            if k2 % 2 == 0:
                xs_pre[0][k2 // 2] = xtile(0, k2 // 2)
            if k2 == 0:
                # per-tap loads for the first group: the very first matmul
                # only has to wait for a 128KB transfer
                w1v = d_w1[0].rearrange("p (g t m c) -> p g t m c", g=2, t=5, m=4)
                for t in range(5):
                    wt = wp.tile([128, 1024], fp8, tag=f"w1s_{t}",
                                 name=f"w1s_{t}")
                    bulk_eng().dma_start(
                        out=wt.rearrange("p (g c) -> p g c", g=2),
                        in_=w1v[:, :, t])
                    w1s.append(wt)
            else:
                w1g[k2] = wtile(d_w1, k2, [128, 5120], fp8, f"w1g_{k2}")
            if k2 % 2 == 1:
                xs_pre[1][k2 // 2] = xtile(1, k2 // 2)
        w2g = [wtile(d_w2, g, [128, 2560], fp8, f"w2g_{g}") for g in range(2)]
        wca2 = wtile(d_wca2, None, [128, 512], fp8, "wca2")
        wca3 = [wtile(d_wca3, k, [128, 1], bf16, f"wca3_{k}", small=True) for k in range(2)]
        qv = [[wtile(d_qv, c * 2 + k, [128, 1], f32, f"qv_{c}_{k}", small=True)
               for k in range(2)] for c in range(2)]
        b1 = [wtile(d_b1, m, [128, 1], f32, f"b1_{m}", small=True) for m in range(M1)]
        b2 = [wtile(d_b2, m, [128, 1], f32, f"b2_{m}", small=True) for m in range(M2)]
        bd1 = [wtile(d_bd1, m, [128, 1], f32, f"bd1_{m}", small=True) for m in range(MD1)]
        bd2 = [wtile(d_bd2, m, [128, 1], f32, f"bd2_{m}", small=True) for m in range(MD2)]
        vm = [wtile(d_v, k, [128, 2], bf16, f"v_{k}", small=True) for k in range(2)]
        bmlp = wtile(d_bmlp, None, [2, 1], f32, "bmlp", small=True)
        mkp = [wtile(d_mask, p, [1, 2 * Lq], f32, f"mask{p}", small=True)
               for p in range(PAIRS)]

        # ---- persistent activation tiles ----
        # cat = t2 k-tiles packed (g, s, 52): 1 zero pad col each side, fp8
        cat = ap_.tile([128, 2 * SEQ * 52], fp8, tag="cat", name="cat")
        nc.gpsimd.memset(cat, 0.0)
        catv = cat.rearrange("p (g s c) -> p g s c", g=2, s=SEQ)
        # d1: two k-groups of (g, s, 102), fp8, 1 zero pad col each side
        d1g = [ap_.tile([128, 2 * SEQ * 102], fp8, tag=f"d1g{g}", name=f"d1g{g}")
               for g in range(2)]
        for t_ in d1g:
            nc.gpsimd.memset(t_, 0.0)
        d2sb = [ap_.tile([128, SEQ * 200], bf16, tag=f"d2_{m}", name=f"d2_{m}") for m in range(MD2)]
        rcol = [[ap_.tile([128, SEQ], f32, tag=f"rcol{c}{k}", name=f"rcol{c}{k}") for k in range(2)]
                for c in range(2)]

        wd1g = [None]
        wcorr = [None] * 8
        wd2g = [None] * 2
        wsum = [None] * 8

        # ============ conv + attention (incl. softmax + r), per pair ========
        for p in range(PAIRS):
            if p >= 1 and p + 1 < PAIRS:
                xs_pre[p + 1] = [xtile(p + 1, k4) for k4 in range(4)]
            if p == 2:
                # deconv weights are needed only after the conv phase; issue
                # them here so they queue behind the x prefetches they'd
                # otherwise starve
                wd1g[0] = wtile(d_wd1, None, [128, 4096], fp8, "wd1g")
                for k in range(2):
                    wd2g[k] = wtile(d_wd2, k, [128, 2048], fp8, f"wd2g_{k}")
                for j in range(8):
                    wcorr[j] = wtile(d_wcorr, j, [128, 512], bf16, f"wcorr_{j}")
                for j in range(8):
                    wsum[j] = wtile(d_wsum, j, [128, 512], bf16, f"wsum_{j}")
            xs = xs_pre[p]

            # conv1: fp8 DoubleRow over 8 double-k groups x 5 taps; k outer /
            # m inner -> each weight tile is consumed for all 4 output tiles
            # as soon as it lands (head-of-kernel overlap)
            psg = [pp.tile([128, 400], f32, tag="mm400", name="mm400", bufs=4)
                   for _ in range(M1)]
            n = 0
            for k2 in range(8):
                k4, kb = divmod(k2, 2)
                rv = xs[k4].rearrange("q (b g s c) -> q b g s c", b=2, g=2, s=2)
                for t in range(5):
                    if k2 == 0:
                        lh = w1s[t].rearrange("p (g m c) -> p g m c", g=2, m=4)
                    else:
                        lh = w1g[k2].rearrange("p (g t m c) -> p g t m c",
                                               g=2, t=5, m=4)[:, :, t]
                    for m in range(M1):
                        nc.tensor.matmul(
                            psg[m], lhsT=lh[:, :, m, :],
                            rhs=rv[:, kb, :, :, t:t + 200],
                            start=(n == 0), stop=(n == 39), perf_mode=DR)
                    n += 1
            t1g = []
            for g in range(2):
                t1t = tp.tile([128, 2 * 2 * 104], fp8, tag=f"t1_{g}",
                              name=f"t1_{g}", bufs=2)
                nc.gpsimd.memset(t1t, 0.0)
                t1g.append(t1t)
            for m in range(M1):
                ps = psg[m]
                tmp = tp.tile([128, 200], f32, tag="ptmp1", name="ptmp1", bufs=3)
                pr = ps.rearrange("q (s l two) -> q s l two", s=2, two=2)
                tv = tmp.rearrange("q (s l) -> q s l", s=2)
                # pool+bias: max(even+b, odd+b); only one PSUM input per inst
                nc.scalar.activation(out=tv, in_=pr[:, :, :, 0],
                                     func=AF.Identity, bias=b1[m], scale=1.0)
                dst = t1g[m // 2].rearrange("p (g s c) -> p g s c", g=2, s=2)[
                    :, m % 2, :, 2:102]
                nc.vector.scalar_tensor_tensor(
                    out=dst, in0=pr[:, :, :, 1], scalar=b1[m], in1=tv,
                    op0=ALU.add, op1=ALU.max)

            # conv2 + pool -> t2 part of cat (psum is 256x; x1/256 on ACT)
            for m in range(M2):
                ps = pp.tile([128, 200], f32, tag="mm200", name="mm200", bufs=2)
                n = 0
                for g in range(2):
                    rv = t1g[g].rearrange("p (k s c) -> p k s c", k=2, s=2)
                    wv = w2g[g].rearrange("p (k t m c) -> p k t m c", k=2, t=5, m=2)
                    for t in range(5):
                        nc.tensor.matmul(
                            ps, lhsT=wv[:, :, t, m, :],
                            rhs=rv[:, :, :, t:t + 100],
                            start=(n == 0), stop=(n == 9), perf_mode=DR)
                        n += 1
                tve = tp.tile([128, 100], f32, tag="ptmp2e", name="ptmp2e", bufs=2)
                tvo = tp.tile([128, 100], f32, tag="ptmp2o", name="ptmp2o", bufs=2)
                pr = ps.rearrange("q (s l two) -> q s l two", s=2, two=2)
                te = tve.rearrange("q (s l) -> q s l", s=2)
                to = tvo.rearrange("q (s l) -> q s l", s=2)
                nc.scalar.activation(out=te, in_=pr[:, :, :, 0],
                                     func=AF.Identity, bias=b2[m], scale=1.0 / 256)
                nc.scalar.activation(out=to, in_=pr[:, :, :, 1],
                                     func=AF.Identity, bias=b2[m], scale=1.0 / 256)
                nc.vector.tensor_max(catv[:, m, 2 * p:2 * p + 2, 1:1 + Lq], te, to)

            # attention for this pair (runs on ACT/DVE/DMA under the next
            # pair's conv1 on PE); kproj psum is 16x
            kp = []
            wcv = wca2.rearrange("p (k c) -> p k c", k=2)
            for m in range(M2):
                kpm = pp.tile([128, 100], f32, tag="mm200", name="mm200", bufs=2)
                nc.tensor.matmul(
                    kpm, lhsT=wcv[:, :, m * 128:(m + 1) * 128],
                    rhs=catv[:, :, 2 * p:2 * p + 2, 1:1 + Lq],
                    start=True, stop=True, perf_mode=DR)
                kp.append(kpm)
            for c in range(2):
                th = []
                for m in range(M2):
                    thm = tp.tile([128, 100], bf16, tag=f"th{c}{m}", name=f"th{c}{m}", bufs=2)
                    nc.scalar.activation(out=thm, in_=kp[m], func=AF.Tanh,
                                         bias=qv[c][m], scale=1.0 / 16)
                    th.append(thm)
                sp = pp.tile([1, 100], f32, tag="tiny", name="tiny", bufs=1)
                for m in range(M2):
                    nc.tensor.matmul(sp, lhsT=wca3[m], rhs=th[m],
                                     start=(m == 0), stop=(m == 1))
                # masked softmax in flat [1, 100] layout (2 blocks of 50);
                # per-block broadcasts use 0-stride AP reads on DVE
                def bc2(t):
                    return bass.AP(tensor=t.tensor, offset=t.offset,
                                   ap=[t.ap[0], [1, 2], [0, Lq]])
                sfl = tp.tile([1, 100], f32, tag="sfl", name="sfl", bufs=4)
                nc.vector.tensor_copy(out=sfl, in_=sp[0:1, 0:100])
                sm = tp.tile([1, 100], f32, tag="sm", name="sm", bufs=4)
                nc.vector.tensor_add(sm, sfl, mkp[p])
                smv = sm.rearrange("q (s l) -> q s l", s=2)
                mx = tp.tile([1, 2], f32, tag="mx", name="mx", bufs=4)
                nc.vector.tensor_reduce(out=mx, in_=smv,
                                        axis=mybir.AxisListType.X, op=ALU.max)
                sub = tp.tile([1, 100], f32, tag="sub", name="sub", bufs=4)
                nc.vector.tensor_sub(sub.rearrange("q (s l) -> q s l", s=2),
                                     smv, bc2(mx))
                ex = tp.tile([1, 100], f32, tag="ex", name="ex", bufs=4)
                nc.scalar.activation(out=ex, in_=sub, func=AF.Exp,
                                     bias=0.0, scale=1.0)
                exv = ex.rearrange("q (s l) -> q s l", s=2)
                se = tp.tile([1, 2], f32, tag="se", name="se", bufs=4)
                nc.vector.tensor_reduce(out=se, in_=exv,
                                        axis=mybir.AxisListType.X, op=ALU.add)
                rc = tp.tile([1, 2], f32, tag="rc", name="rc", bufs=4)
                nc.vector.reciprocal(rc, se)
                av = tp.tile([1, 100], f32, tag="av", name="av", bufs=4)
                nc.vector.tensor_mul(av.rearrange("q (s l) -> q s l", s=2),
                                     exv, bc2(rc))
                # broadcast attention weights to all partitions (no DRAM hop)
                abc2 = tp.tile([128, 100], f32, tag="abc", name="abc", bufs=4)
                nc.gpsimd.partition_broadcast(abc2, av[0:1, :], channels=128)
                for s01 in range(2):
                    s = 2 * p + s01
                    abc = abc2[:, Lq * s01:Lq * s01 + Lq]
                    for k in range(2):
                        scr = tp.tile([128, Lq], f32, tag="rscr", name="rscr", bufs=2)
                        nc.vector.tensor_mul(
                            scr, catv[:, k, s, 1:1 + Lq], abc)
                        nc.vector.tensor_reduce(
                            out=rcol[c][k][:, s:s + 1], in_=scr,
                            axis=mybir.AxisListType.X, op=ALU.add)

        # ================= deconv1: cat[768,50] -> d1[512,100] ==============
        # even out o=2j: tap1@U[j] + tap3@U[j-1]; odd o=2j+1: tap2@U[j] + tap0@U[j+1]
        # r-channels (256..767) are constant along j, so their contribution is
        # rank-1 per sequence: broadcast((Wt_a+Wt_b)^T r) with a single-column
        # boundary correction (j=0 even: -W3^T r; j=49 odd: -W0^T r).
        rcolb = []
        for c in range(2):
            for k in range(2):
                rb = ap_.tile([128, SEQ], bf16, tag=f"rcolb{c}{k}",
                              name=f"rcolb{c}{k}")
                nc.vector.tensor_copy(out=rb, in_=rcol[c][k])
                rcolb.append(rb)          # rk = 2*c + k matches cat channel order
        rt = pp.tile([128, 320], f32, tag="rt", name="rt", bufs=1)
        for phase in range(2):
            for m in range(MD1):
                g = phase * 4 + m
                for rk in range(4):
                    nc.tensor.matmul(
                        rt[:, g * 20:g * 20 + 10],
                        lhsT=wsum[phase * 4 + rk][:, m * 128:(m + 1) * 128],
                        rhs=rcolb[rk], start=(rk == 0), stop=(rk == 3))
                for rk in range(4):
                    nc.tensor.matmul(
                        rt[:, g * 20 + 10:g * 20 + 20],
                        lhsT=wcorr[phase * 4 + rk][:, m * 128:(m + 1) * 128],
                        rhs=rcolb[rk], start=(rk == 0), stop=(rk == 3))
        wd1v = wd1g[0].rearrange("p (k t m c) -> p k t m c", k=2, t=4, m=4)
        for m in range(MD1):
            for phase, taps in enumerate([[(1, 1), (3, 0)], [(2, 1), (0, 2)]]):
                ps = pp.tile([128, SEQ * Lq], f32, tag="mm400", name="mm400", bufs=4)
                n = 0
                for t, off in taps:
                    nc.tensor.matmul(
                        ps, lhsT=wd1v[:, :, t, m, :],
                        rhs=catv[:, :, :, off:off + Lq],
                        start=(n == 0), stop=(n == 1), perf_mode=DR)
                    n += 1
                g = phase * 4 + m
                ert = tp.tile([128, 20], f32, tag="ert", name="ert", bufs=2)
                nc.vector.tensor_copy(out=ert, in_=rt[:, g * 20:g * 20 + 20])
                er2 = tp.tile([128, SEQ], f32, tag="er2", name="er2", bufs=2)
                nc.vector.tensor_sub(er2, ert[:, 0:10], ert[:, 10:20])
                # psum is 16x: divide back out + bias on ACT, then add the
                # rank-1 r contribution on DVE and store fp8
                tv = tp.tile([128, SEQ * Lq], f32, tag="dtv", name="dtv", bufs=2)
                nc.scalar.activation(out=tv, in_=ps, func=AF.Identity,
                                     bias=bd1[m], scale=1.0 / 16)
                tvv = tv.rearrange("q (s l) -> q s l", s=SEQ)
                dst = d1g[m // 2].rearrange("p (k s c) -> p k s c", k=2, s=SEQ)[
                    :, m % 2, :, 1:101] \
                    .rearrange("q s (l two) -> q s l two", two=2)[:, :, :, phase]
                erb = bass.AP(tensor=ert.tensor, offset=ert.offset,
                              ap=[ert.ap[0], [1, SEQ], [0, Lq]])
                nc.vector.tensor_add(dst, tvv, erb)
                bcol = 0 if phase == 0 else Lq - 1
                nc.vector.tensor_add(
                    dst[:, :, bcol:bcol + 1], tvv[:, :, bcol:bcol + 1],
                    er2.rearrange("q (s o) -> q s o", o=1))

        # ================= deconv2: d1[512,100] -> d2[256,200] ==============
        for m in range(MD2):
            for g5 in range(2):         # groups of 5 seqs (PSUM bank limit)
                for phase, taps in enumerate([[(1, 1), (3, 0)], [(2, 1), (0, 2)]]):
                    ps = pp.tile([128, 500], f32, tag="mm400", name="mm400", bufs=4)
                    n = 0
                    for kg in range(2):
                        dv = d1g[kg].rearrange("p (k s c) -> p k s c", k=2, s=SEQ)
                        wv = wd2g[kg].rearrange("p (k t m c) -> p k t m c",
                                                k=2, t=4, m=2)
                        for t, off in taps:
                            nc.tensor.matmul(
                                ps, lhsT=wv[:, :, t, m, :],
                                rhs=dv[:, :, 5 * g5:5 * g5 + 5, off:off + 100],
                                start=(n == 0), stop=(n == 3), perf_mode=DR)
                            n += 1
                    dst = d2sb[m].rearrange("q (s c) -> q s c", s=SEQ)[
                        :, 5 * g5:5 * g5 + 5, :] \
                        .rearrange("q s (l two) -> q s l two", two=2)[:, :, :, phase]
                    nc.scalar.activation(
                        out=dst, in_=ps.rearrange("q (s l) -> q s l", s=5),
                        func=AF.Identity, bias=bd2[m], scale=1.0 / 16)

        # ================= folded final projection + sigmoid ================
        for p in range(PAIRS):
            fp = pp.tile([2, 400], f32, tag="tiny", name="tiny", bufs=1)
            for k in range(2):
                nc.tensor.matmul(fp, lhsT=vm[k],
                                 rhs=d2sb[k][:, 400 * p:400 * (p + 1)],
                                 start=(k == 0), stop=(k == 1))
            fo = tp.tile([2, 400], f32, tag="fout", name="fout", bufs=5)
            nc.scalar.activation(out=fo, in_=fp, func=AF.Sigmoid,
                                 bias=bmlp, scale=1.0)
            nc.sync.dma_start(out=d_out[:, 400 * p:400 * (p + 1)], in_=fo)

    nc.compile()   # bacc legalization: splits sync waits to <=1 per inst
    return nc


def _prep_inputs(batch, seg_len, concept1, concept2,
                 w_conv1, b_conv1, w_conv2, b_conv2,
                 w_ca1, w_ca2, w_ca3,
                 w_dc1, b_dc1, w_dc2, b_dc2,
                 w_sim1, w_sim2, w_mlp, b_mlp):
    f32 = np.float32

    # x: [B,M,L,IN_C] -> per core [PAIRS*4, 128, 1632] fp8: (k2b, g, s, 204)
    bm = np.ascontiguousarray(batch, f32).reshape(B * M, L, IN_C)
    bt = _q8(bm.transpose(0, 2, 1))                    # [80, 2048, 200]
    X = np.zeros((B * M, 16, 128, 204), E4M3)
    X[:, :, :, 2:202] = bt.reshape(B * M, 16, 128, L)
    xw = X.reshape(NCORES, PAIRS, 2, 4, 2, 2, 128, 204) \
          .transpose(0, 1, 3, 6, 4, 5, 2, 7) \
          .reshape(NCORES, PAIRS * 4, 128, 1632)
    xw = np.ascontiguousarray(xw)

    # conv weights, 16x, fp8, DoubleRow layout [k-group, ci, (g, t, m, co)]
    w1t = np.ascontiguousarray(
        _q8(SW * np.asarray(w_conv1, f32)).reshape(M1, 128, 8, 2, 128, 5)
        .transpose(2, 4, 3, 5, 0, 1).reshape(8, 128, 5120))
    w2t = np.ascontiguousarray(
        _q8(SW * np.asarray(w_conv2, f32)).reshape(M2, 128, 2, 2, 128, 5)
        .transpose(2, 4, 3, 5, 0, 1).reshape(2, 128, 2560))
    wd1t = np.ascontiguousarray(
        _q8(SW * np.asarray(w_dc1, f32)[:256]).reshape(2, 128, MD1, 128, 4)
        .transpose(1, 0, 4, 2, 3).reshape(128, 4096))
    wd2t = np.ascontiguousarray(
        _q8(SW * np.asarray(w_dc2, f32)).reshape(2, 2, 128, MD2, 128, 4)
        .transpose(0, 2, 1, 5, 3, 4).reshape(2, 128, 2048))
    wca2t = np.ascontiguousarray(
        _q8(SW * np.asarray(w_ca2, f32)).T.reshape(2, 128, 256)
        .transpose(1, 0, 2).reshape(128, 512))
    # summed-tap deconv1 weights for the broadcast r-channels: [ph*4+rk, ci, (m co)]
    wr = np.asarray(w_dc1, f32)[256:768].reshape(4, 128, MD1, 128, 4)
    wsum = np.ascontiguousarray(
        np.stack([wr[..., 1] + wr[..., 3], wr[..., 2] + wr[..., 0]], 0)
        .reshape(8, 128, 512)).astype(BF16)
    # boundary-correction taps: phase 0 -> tap 3, phase 1 -> tap 0
    wcorr = np.ascontiguousarray(
        np.stack([wr[..., 3], wr[..., 0]], 0).reshape(8, 128, 512)).astype(BF16)
    wca3t = np.asarray(w_ca3, f32)[0].reshape(2, 128, 1).astype(BF16)
    b1 = SW * np.asarray(b_conv1, f32).reshape(M1, 128, 1)
    b2 = np.asarray(b_conv2, f32).reshape(M2, 128, 1)
    bd1v = np.asarray(b_dc1, f32).reshape(MD1, 128, 1)
    bd2v = np.asarray(b_dc2, f32).reshape(MD2, 128, 1)
    bmlp = np.full((2, 1), np.asarray(b_mlp, f32).reshape(-1)[0], f32)

    # per-core mask / q / v
    nvalid = ((np.asarray(seg_len) + 3) // 4).reshape(B * M)
    amask = np.where(np.arange(Lq)[None, :] < nvalid[:, None], 0.0, NEG) \
        .astype(f32).reshape(NCORES, PAIRS, 1, 2 * Lq)
    concepts = [np.asarray(concept1, f32), np.asarray(concept2, f32)]
    w_ca1 = np.asarray(w_ca1, f32)
    w_sim1 = np.asarray(w_sim1, f32)
    w_sim2 = np.asarray(w_sim2, f32)
    wm = np.asarray(w_mlp, f32)[0]
    qv_all = np.zeros((NCORES, 4, 128, 1), f32)
    v_all = np.zeros((NCORES, 2, 128, 2), f32)
    for core in range(NCORES):
        bidx = (core * SEQ) // M
        for c in range(2):
            q = w_ca1 @ concepts[c][bidx]                       # [256]
            qv_all[core, 2 * c:2 * c + 2] = q.reshape(2, 128, 1)
            v = w_sim1.T @ ((w_sim2 @ concepts[c][bidx]) * wm)  # [256]
            v_all[core, :, :, c] = v.reshape(2, 128)
    vmat = v_all.astype(BF16)

    shared = dict(w1t=w1t, b1=b1, wsum=wsum, wcorr=wcorr, w2t=w2t, b2=b2,
                  wca2t=wca2t, wca3t=wca3t,
                  wd1t=wd1t, bd1=bd1v, wd2t=wd2t, bd2=bd2v, bmlp=bmlp)
    return [dict(shared, xw=xw[c], amask=amask[c], qv=qv_all[c], vmat=vmat[c])
            for c in range(NCORES)]


_CACHE = {}


def kernel(**inputs):
    from concourse.bass_utils import run_bass_kernel_spmd

    in_maps = _prep_inputs(**inputs)
    if "nc" not in _CACHE:
        _CACHE["nc"] = _build_program()
    res = run_bass_kernel_spmd(_CACHE["nc"], in_maps, list(range(NCORES)))
    out = np.stack([np.asarray(r["out"], np.float32) for r in res.results])
    sc = out.transpose(1, 0, 2).reshape(2, B, M, L)
    return sc[0], sc[1]
